# revision 2
# baseline (speedup 1.0000x reference)
"""Trainium2 Bass kernel for nn_AutomatonNetwork (V=128 symbols, N=512 states,
L=4096 tokens, 8 NeuronCores).

Reference computes, sequentially over t:
    prob *= internal @ probs[tokens[t]]
    internal = internal @ transfer[tokens[t]]
then prob *= internal @ finals; returns (internal, prob).

Strategy
--------
The transfer matrices are row-stochastic, so products of them contract the
simplex exponentially fast (~15x error reduction per step on this data). A
16-step warm-up walk re-synchronizes a chunk's state to f32 precision from
*any* starting distribution, which makes the 4096-step chain splittable into
independent chunks: 8 cores x 2 interleaved chains of 256 steps, each chain
preceded by a 16-step warm-up over the preceding tokens (core 0's first
chain warms up on identity matrices so it starts exactly at `start`). Two
chains per core keep the tensor engine busy while the other chain's
state-vector copy + semaphore round-trip completes.

Each chain step is a vector x matrix product on the tensor engine
(T-stationary: lhsT = the transfer matrix in natural layout as 16 [128,128]
tiles, rhs = the state as a [128,4] column group, f32 PSUM accumulation).
Every intermediate state is recorded to SBUF and the full f32 history is
DMA-ed out at the end. Matrices stream from HBM as host-pre-gathered
reduced-precision streams in partition-major layout (each partition reads
one contiguous run per DMA group), keeping DMA near peak HBM bandwidth.

Precision: matrices in fp8-e4m3 scaled by 64 (the scale moves the ~1/N-size
entries out of the fp8 subnormal range; the state picks up a constant
per-step factor that the host normalization absorbs), except the last 16
steps of the final chain, which run in fp16 so the *final* internal state
has fp16-level fidelity. AUTOMATON_DTYPE=float16/float32 selects uniform
higher-precision variants instead.

Host post-processing: the emission dots d_t = v_t . probs[tokens[t]] and
their product are computed on the host in float64 from the recorded state
history. Because the product of 4096 near-1 factors amplifies any
systematic error by 4096x, each recorded state is renormalized and the true
sum is propagated analytically: sum(v_{t+1}) = sum_k vhat_k rowsum(T_t)_k,
with rowsums from the ORIGINAL f32 transfer bank in float64. This removes
matrix-rounding drift, device accumulation bias, and all scale factors,
leaving ~1e-4 relative error on prob (the f32 reference's own rounding
level, measured in simulation for fp8/fp16/f32 alike).
"""

import os

import numpy as np

V = 128
N = 512
L = 4096
NCORES = 8
KWARM = 16
NCHAINS = 2
CH = L // NCORES                  # 512 tokens per core
CL = CH // NCHAINS                # 256 main steps per chain
SS = KWARM + CL                   # 272 steps per chain
GRP = 8                           # steps per DMA group
KC = N // 128                     # 4 contraction chunks
FAT = KC * N                      # 2048 elements per fat row
TAIL = 16                         # fp16 tail steps (fp8 mode only)
ST = 64.0                         # fp8 matrix scale
SV = 512.0                        # fp8 state scale

_compiled = {}
last_results = None


def _mdt(name):
    from concourse import mybir
    return {"float16": mybir.dt.float16, "float32": mybir.dt.float32,
            "fp8": mybir.dt.float8e4}[name]


def _build_program(mode):
    """SPMD Bass/Tile program (token-independent; all per-core variation is
    input data). mode in {'fp8','float16','float32'}."""
    import concourse.tile as tile
    from concourse import bacc, mybir

    mdt = _mdt(mode)
    grp = GRP if mode != "float32" else GRP // 2
    ngrp = SS // grp
    assert SS % grp == 0

    nc = bacc.Bacc("TRN2", target_bir_lowering=False, debug=False,
                   enable_asserts=False, num_devices=NCORES)
    mats = nc.dram_tensor("mats", [128, NCHAINS * SS, FAT], mdt,
                          kind="ExternalInput").ap()
    v0 = nc.dram_tensor("v0", [128, KC], mybir.dt.float32,
                        kind="ExternalInput").ap()
    if mode == "fp8":
        tailm = nc.dram_tensor("tailm", [128, TAIL, FAT], mybir.dt.float16,
                               kind="ExternalInput").ap()
    vhist_out = nc.dram_tensor("vhist", [128, NCHAINS * (SS + 1) * KC],
                               mybir.dt.float32, kind="ExternalOutput").ap()

    def step_dtype(q, s):
        """dtype of the matrices/state used by step s of chain q."""
        if mode == "fp8" and q == NCHAINS - 1 and s >= SS - TAIL:
            return "f16tail"
        return mode

    with tile.TileContext(nc) as tc:
        with (
            tc.tile_pool(name="mats", bufs=3) as mats_pool,
            tc.tile_pool(name="tail", bufs=1) as tail_pool,
            tc.tile_pool(name="vcur", bufs=4) as vcur_pool,
            tc.tile_pool(name="hist", bufs=1) as hist_pool,
            tc.tile_pool(name="psum", bufs=3, space="PSUM") as psum_pool,
        ):
            vhist = hist_pool.tile([128, NCHAINS * (SS + 1) * KC],
                                   mybir.dt.float32)
            v0sb = vcur_pool.tile([128, KC], mybir.dt.float32, tag="v0")
            nc.sync.dma_start(v0sb[:], v0[:])

            tail_sb = None
            if mode == "fp8":
                tail_sb = tail_pool.tile([128, TAIL, FAT], mybir.dt.float16)
                nc.sync.dma_start(tail_sb[:], tailm[:])

            # initial vcur per chain (cast of v0; fp8 mode input is
            # pre-scaled by SV on the host)
            vcur = []
            for q in range(NCHAINS):
                dt0 = _mdt("float16" if step_dtype(q, 0) == "f16tail"
                           else step_dtype(q, 0))
                vc = vcur_pool.tile([128, KC], dt0, tag=f"vcur{q}")
                nc.vector.tensor_copy(vc[:], v0sb[:])
                nc.scalar.copy(
                    vhist[:, (q * (SS + 1)) * KC:(q * (SS + 1) + 1) * KC],
                    v0sb[:])
                vcur.append(vc)

            mg = [None] * NCHAINS
            for s in range(SS):
                g, j = divmod(s, grp)
                for q in range(NCHAINS):
                    if j == 0:
                        mg[q] = mats_pool.tile([128, grp, FAT], mdt,
                                               tag=f"mats{q}")
                        nc.sync.dma_start(
                            mg[q][:],
                            mats[:, q * SS + g * grp:q * SS + (g + 1) * grp, :])
                    sd = step_dtype(q, s)
                    if sd == "f16tail":
                        src = tail_sb
                        off = s - (SS - TAIL)
                    else:
                        src = mg[q]
                        off = j
                    ps = psum_pool.tile([128, KC], mybir.dt.float32,
                                        tag=f"ps{q}")
                    for mb in range(KC):
                        for kc in range(KC):
                            nc.tensor.matmul(
                                out=ps[:, mb:mb + 1],
                                lhsT=src[:, off,
                                         kc * N + mb * 128:kc * N + (mb + 1) * 128],
                                rhs=vcur[q][:, kc:kc + 1],
                                start=(kc == 0),
                                stop=(kc == KC - 1),
                            )
                    # state copy for the next step, in that step's dtype
                    if s + 1 < SS:
                        nd = step_dtype(q, s + 1)
                        if nd == "f16tail":
                            vc = vcur_pool.tile([128, KC], mybir.dt.float16,
                                                tag=f"vcur{q}")
                            nc.vector.tensor_copy(vc[:], ps[:])
                        elif mode == "fp8":
                            vc = vcur_pool.tile([128, KC], mdt,
                                                tag=f"vcur{q}")
                            nc.vector.tensor_scalar_mul(vc[:], ps[:], 1.0 / ST)
                        else:
                            vc = vcur_pool.tile([128, KC], mdt,
                                                tag=f"vcur{q}")
                            nc.vector.tensor_copy(vc[:], ps[:])
                        vcur[q] = vc
                    nc.scalar.copy(
                        vhist[:, (q * (SS + 1) + s + 1) * KC:
                              (q * (SS + 1) + s + 2) * KC],
                        ps[:])

            nc.sync.dma_start(vhist_out[:], vhist[:])
    nc.compile()
    return nc


def _fatten(mat_block):
    """[..., N, N] -> [..., 128, FAT]: fat[p, kc*N+m] = M[kc*128+p, m]."""
    shp = mat_block.shape[:-2]
    return (mat_block.reshape(shp + (KC, 128, N))
            .swapaxes(-3, -2)
            .reshape(shp + (128, FAT)))


def _np_dt(mode):
    if mode == "fp8":
        from concourse import mybir
        return mybir.dt.np(mybir.dt.float8e4)
    return {"float16": np.float16, "float32": np.float32}[mode]


def kernel(tokens, start, transfer, probs, finals):
    global last_results
    from concourse.bass_utils import run_bass_kernel_spmd

    tokens = np.asarray(tokens)
    start = np.asarray(start, dtype=np.float32)
    transfer = np.asarray(transfer, dtype=np.float32)
    probs = np.asarray(probs, dtype=np.float32)
    finals = np.asarray(finals, dtype=np.float32)

    mode = os.environ.get("AUTOMATON_DTYPE", "fp8")
    np_dt = _np_dt(mode)
    mscale = ST if mode == "fp8" else 1.0
    vscale = SV if mode == "fp8" else 1.0

    # ---- host-side input prep ----
    fat_bank = _fatten((transfer * mscale).astype(np_dt))   # [V, 128, FAT]
    ident = np.zeros((128, FAT), dtype=np.float32)
    for kc in range(KC):
        ident[np.arange(128), kc * N + kc * 128 + np.arange(128)] = mscale
    ident = ident.astype(np_dt)

    v0sb = np.ascontiguousarray(
        (start * vscale).reshape(KC, 128).T).astype(np.float32)

    if mode == "fp8":
        fat16 = None  # built lazily per needed symbol
        tail_toks = tokens[L - TAIL:]
        tail_np = _fatten(transfer[tail_toks].astype(np.float16))
        tailm = np.ascontiguousarray(tail_np.transpose(1, 0, 2))  # [128,TAIL,FAT]

    in_maps = []
    for c in range(NCORES):
        stream = np.empty((NCHAINS * SS, 128, FAT), dtype=np_dt)
        for q in range(NCHAINS):
            t0 = c * CH + q * CL
            if t0 == 0:
                stream[q * SS:q * SS + KWARM] = ident[None]
            else:
                stream[q * SS:q * SS + KWARM] = fat_bank[tokens[t0 - KWARM:t0]]
            stream[q * SS + KWARM:(q + 1) * SS] = fat_bank[tokens[t0:t0 + CL]]
        mats_c = np.ascontiguousarray(stream.transpose(1, 0, 2))
        m = {"mats": mats_c, "v0": v0sb}
        if mode == "fp8":
            m["tailm"] = tailm
        in_maps.append(m)

    # ---- compile (cached in-process) + run ----
    if mode not in _compiled:
        _compiled[mode] = _build_program(mode)
    nc = _compiled[mode]
    last_results = run_bass_kernel_spmd(nc, in_maps,
                                        core_ids=list(range(NCORES)))

    # ---- host post-processing ----
    internals = np.empty((L, N), dtype=np.float64)
    for c in range(NCORES):
        h = last_results.results[c]["vhist"].reshape(128, NCHAINS, SS + 1, KC)
        # v_s[kc*128+p] = h[p, q, s, kc]
        vs = np.ascontiguousarray(h.transpose(1, 2, 3, 0)) \
            .reshape(NCHAINS, SS + 1, N)
        for q in range(NCHAINS):
            t0 = c * CH + q * CL
            internals[t0:t0 + CL] = vs[q, KWARM:KWARM + CL]
        if c == NCORES - 1:
            vfin = vs[NCHAINS - 1, SS].astype(np.float64)

    rowsum = transfer.astype(np.float64).sum(axis=2)
    vn = internals / internals.sum(axis=1)[:, None]
    ratios = np.einsum("tn,tn->t", vn, rowsum[tokens])
    s0 = float(start.astype(np.float64).sum())
    scum = s0 * np.concatenate(([1.0], np.cumprod(ratios)))
    d = np.einsum("tn,tn->t", vn, probs[tokens].astype(np.float64)) * scum[:L]
    vfin_c = vfin / vfin.sum() * scum[L]
    prob = np.prod(d) * float(vfin_c @ finals.astype(np.float64))

    return vfin_c.astype(np.float32), np.float32(prob)


# revision 3
# speedup vs baseline: 1.4809x; 1.4809x over previous
"""Trainium2 Bass kernel for nn_AutomatonNetwork (V=128 symbols, N=512 states,
L=4096 tokens, 8 NeuronCores).

Reference computes, sequentially over t:
    prob *= internal @ probs[tokens[t]]
    internal = internal @ transfer[tokens[t]]
then prob *= internal @ finals; returns (internal, prob).

Strategy
--------
The transfer matrices are row-stochastic, so products of them contract the
simplex exponentially fast (~15x error reduction per step on this data). A
16-step warm-up walk re-synchronizes a chunk's state to f32 precision from
*any* starting distribution, which makes the 4096-step chain splittable into
independent chunks: 8 cores x 2 interleaved chains of 256 steps, each chain
preceded by a 16-step warm-up over the preceding tokens (core 0's first
chain warms up on identity matrices so it starts exactly at `start`). Two
chains per core keep the tensor engine busy while the other chain's
state-vector copy + semaphore round-trip completes.

Each chain step is a vector x matrix product on the tensor engine
(T-stationary: lhsT = the transfer matrix in natural layout as 16 [128,128]
tiles, rhs = the state as a [128,4] column group, f32 PSUM accumulation).
Every intermediate state is recorded to SBUF and the full f32 history is
DMA-ed out at the end. Matrices stream from HBM as host-pre-gathered
reduced-precision streams in partition-major layout (each partition reads
one contiguous run per DMA group), keeping DMA near peak HBM bandwidth.

Precision: matrices in fp8-e4m3 scaled by 64 (the scale moves the ~1/N-size
entries out of the fp8 subnormal range; the state picks up a constant
per-step factor that the host normalization absorbs), except the last 16
steps of the final chain, which run in fp16 so the *final* internal state
has fp16-level fidelity. AUTOMATON_DTYPE=float16/float32 selects uniform
higher-precision variants instead.

Host post-processing: the emission dots d_t = v_t . probs[tokens[t]] and
their product are computed on the host in float64 from the recorded state
history. Because the product of 4096 near-1 factors amplifies any
systematic error by 4096x, each recorded state is renormalized and the true
sum is propagated analytically: sum(v_{t+1}) = sum_k vhat_k rowsum(T_t)_k,
with rowsums from the ORIGINAL f32 transfer bank in float64. This removes
matrix-rounding drift, device accumulation bias, and all scale factors,
leaving ~1e-4 relative error on prob (the f32 reference's own rounding
level, measured in simulation for fp8/fp16/f32 alike).
"""

import os

import numpy as np

V = 128
N = 512
L = 4096
NCORES = 8
KWARM = 16
NCHAINS = 2
CH = L // NCORES                  # 512 tokens per core
CL = CH // NCHAINS                # 256 main steps per chain
SS = KWARM + CL                   # 272 steps per chain
GRP = 8                           # steps per DMA group
KC = N // 128                     # 4 contraction chunks
FAT = KC * N                      # 2048 elements per fat row
TAIL = 16                         # fp16 tail steps (fp8 mode only)
ST = 64.0                         # fp8 matrix scale
SV = 512.0                        # fp8 state scale

_compiled = {}
last_results = None


def _mdt(name):
    from concourse import mybir
    return {"float16": mybir.dt.float16, "float32": mybir.dt.float32,
            "fp8": mybir.dt.float8e4}[name]


def _build_program(mode):
    """SPMD Bass/Tile program (token-independent; all per-core variation is
    input data). mode in {'fp8','float16','float32'}."""
    import concourse.tile as tile
    from concourse import bacc, mybir

    mdt = _mdt(mode)
    grp = GRP if mode != "float32" else GRP // 2
    ngrp = SS // grp
    assert SS % grp == 0

    nc = bacc.Bacc("TRN2", target_bir_lowering=False, debug=False,
                   enable_asserts=False, num_devices=NCORES)
    mats = nc.dram_tensor("mats", [128, NCHAINS * SS, FAT], mdt,
                          kind="ExternalInput").ap()
    v0 = nc.dram_tensor("v0", [128, KC], mybir.dt.float32,
                        kind="ExternalInput").ap()
    if mode == "fp8":
        tailm = nc.dram_tensor("tailm", [128, TAIL, FAT], mybir.dt.float16,
                               kind="ExternalInput").ap()
    vhist_out = nc.dram_tensor("vhist", [128, NCHAINS * (SS + 1) * KC],
                               mybir.dt.float32, kind="ExternalOutput").ap()

    def step_dtype(q, s):
        """dtype of the matrices/state used by step s of chain q."""
        if mode == "fp8" and q == NCHAINS - 1 and s >= SS - TAIL:
            return "f16tail"
        return mode

    with tile.TileContext(nc) as tc:
        with (
            tc.tile_pool(name="mats", bufs=3) as mats_pool,
            tc.tile_pool(name="tail", bufs=1) as tail_pool,
            tc.tile_pool(name="vcur", bufs=4) as vcur_pool,
            tc.tile_pool(name="hist", bufs=1) as hist_pool,
            tc.tile_pool(name="psum", bufs=3, space="PSUM") as psum_pool,
        ):
            vhist = hist_pool.tile([128, NCHAINS * (SS + 1) * KC],
                                   mybir.dt.float32)
            v0sb = vcur_pool.tile([128, KC], mybir.dt.float32, tag="v0")
            nc.sync.dma_start(v0sb[:], v0[:])

            tail_sb = None
            if mode == "fp8":
                tail_sb = tail_pool.tile([128, TAIL, FAT], mybir.dt.float16)
                nc.sync.dma_start(tail_sb[:], tailm[:])

            # initial vcur per chain (cast of v0; fp8 mode input is
            # pre-scaled by SV on the host)
            vcur = []
            for q in range(NCHAINS):
                dt0 = _mdt("float16" if step_dtype(q, 0) == "f16tail"
                           else step_dtype(q, 0))
                vc = vcur_pool.tile([128, KC], dt0, tag=f"vcur{q}")
                nc.vector.tensor_copy(vc[:], v0sb[:])
                nc.scalar.copy(
                    vhist[:, (q * (SS + 1)) * KC:(q * (SS + 1) + 1) * KC],
                    v0sb[:])
                vcur.append(vc)

            mg = [None] * NCHAINS
            for s in range(SS):
                g, j = divmod(s, grp)
                for q in range(NCHAINS):
                    if j == 0:
                        mg[q] = mats_pool.tile([128, grp, FAT], mdt,
                                               name=f"mg{q}_{g}",
                                               tag=f"mats{q}")
                        nc.sync.dma_start(
                            mg[q][:],
                            mats[:, q * SS + g * grp:q * SS + (g + 1) * grp, :])
                    sd = step_dtype(q, s)
                    if sd == "f16tail":
                        src = tail_sb
                        off = s - (SS - TAIL)
                    else:
                        src = mg[q]
                        off = j
                    ps = psum_pool.tile([128, KC], mybir.dt.float32,
                                        tag=f"ps{q}")
                    for mb in range(KC):
                        for kc in range(KC):
                            nc.tensor.matmul(
                                out=ps[:, mb:mb + 1],
                                lhsT=src[:, off,
                                         kc * N + mb * 128:kc * N + (mb + 1) * 128],
                                rhs=vcur[q][:, kc:kc + 1],
                                start=(kc == 0),
                                stop=(kc == KC - 1),
                            )
                    # state copy for the next step, in that step's dtype
                    if s + 1 < SS:
                        nd = step_dtype(q, s + 1)
                        if nd == "f16tail":
                            vc = vcur_pool.tile([128, KC], mybir.dt.float16,
                                                tag=f"vcur{q}")
                            nc.vector.tensor_copy(vc[:], ps[:])
                        elif mode == "fp8":
                            vc = vcur_pool.tile([128, KC], mdt,
                                                tag=f"vcur{q}")
                            nc.vector.tensor_scalar_mul(vc[:], ps[:], 1.0 / ST)
                        else:
                            vc = vcur_pool.tile([128, KC], mdt,
                                                tag=f"vcur{q}")
                            nc.vector.tensor_copy(vc[:], ps[:])
                        vcur[q] = vc
                    nc.scalar.copy(
                        vhist[:, (q * (SS + 1) + s + 1) * KC:
                              (q * (SS + 1) + s + 2) * KC],
                        ps[:])

            nc.sync.dma_start(vhist_out[:], vhist[:])
    nc.compile()
    return nc


def _fatten(mat_block):
    """[..., N, N] -> [..., 128, FAT]: fat[p, kc*N+m] = M[kc*128+p, m]."""
    shp = mat_block.shape[:-2]
    return (mat_block.reshape(shp + (KC, 128, N))
            .swapaxes(-3, -2)
            .reshape(shp + (128, FAT)))


def _np_dt(mode):
    if mode == "fp8":
        from concourse import mybir
        return mybir.dt.np(mybir.dt.float8e4)
    return {"float16": np.float16, "float32": np.float32}[mode]


def kernel(tokens, start, transfer, probs, finals):
    global last_results
    from concourse.bass_utils import run_bass_kernel_spmd

    tokens = np.asarray(tokens)
    start = np.asarray(start, dtype=np.float32)
    transfer = np.asarray(transfer, dtype=np.float32)
    probs = np.asarray(probs, dtype=np.float32)
    finals = np.asarray(finals, dtype=np.float32)

    mode = os.environ.get("AUTOMATON_DTYPE", "fp8")
    np_dt = _np_dt(mode)
    mscale = ST if mode == "fp8" else 1.0
    vscale = SV if mode == "fp8" else 1.0

    # ---- host-side input prep ----
    fat_bank = _fatten((transfer * mscale).astype(np_dt))   # [V, 128, FAT]
    ident = np.zeros((128, FAT), dtype=np.float32)
    for kc in range(KC):
        ident[np.arange(128), kc * N + kc * 128 + np.arange(128)] = mscale
    ident = ident.astype(np_dt)

    v0sb = np.ascontiguousarray(
        (start * vscale).reshape(KC, 128).T).astype(np.float32)

    if mode == "fp8":
        fat16 = None  # built lazily per needed symbol
        tail_toks = tokens[L - TAIL:]
        tail_np = _fatten(transfer[tail_toks].astype(np.float16))
        tailm = np.ascontiguousarray(tail_np.transpose(1, 0, 2))  # [128,TAIL,FAT]

    in_maps = []
    for c in range(NCORES):
        stream = np.empty((NCHAINS * SS, 128, FAT), dtype=np_dt)
        for q in range(NCHAINS):
            t0 = c * CH + q * CL
            if t0 == 0:
                stream[q * SS:q * SS + KWARM] = ident[None]
            else:
                stream[q * SS:q * SS + KWARM] = fat_bank[tokens[t0 - KWARM:t0]]
            stream[q * SS + KWARM:(q + 1) * SS] = fat_bank[tokens[t0:t0 + CL]]
        mats_c = np.ascontiguousarray(stream.transpose(1, 0, 2))
        m = {"mats": mats_c, "v0": v0sb}
        if mode == "fp8":
            m["tailm"] = tailm
        in_maps.append(m)

    # ---- compile (cached in-process) + run ----
    if mode not in _compiled:
        _compiled[mode] = _build_program(mode)
    nc = _compiled[mode]
    last_results = run_bass_kernel_spmd(nc, in_maps,
                                        core_ids=list(range(NCORES)))

    # ---- host post-processing ----
    internals = np.empty((L, N), dtype=np.float64)
    for c in range(NCORES):
        h = last_results.results[c]["vhist"].reshape(128, NCHAINS, SS + 1, KC)
        # v_s[kc*128+p] = h[p, q, s, kc]
        vs = np.ascontiguousarray(h.transpose(1, 2, 3, 0)) \
            .reshape(NCHAINS, SS + 1, N)
        for q in range(NCHAINS):
            t0 = c * CH + q * CL
            internals[t0:t0 + CL] = vs[q, KWARM:KWARM + CL]
        if c == NCORES - 1:
            vfin = vs[NCHAINS - 1, SS].astype(np.float64)

    rowsum = transfer.astype(np.float64).sum(axis=2)
    vn = internals / internals.sum(axis=1)[:, None]
    ratios = np.einsum("tn,tn->t", vn, rowsum[tokens])
    s0 = float(start.astype(np.float64).sum())
    scum = s0 * np.concatenate(([1.0], np.cumprod(ratios)))
    d = np.einsum("tn,tn->t", vn, probs[tokens].astype(np.float64)) * scum[:L]
    vfin_c = vfin / vfin.sum() * scum[L]
    prob = np.prod(d) * float(vfin_c @ finals.astype(np.float64))

    return vfin_c.astype(np.float32), np.float32(prob)
